# revision 1
# baseline (speedup 1.0000x reference)
"""Causal multi-head attention (B=4, T=2048, C=1024, 16 heads) on 8 TRN2 NeuronCores.

Sharding: data-parallel over (batch, q-chunk-pair). Core 2*b+h handles batch b
and two 512-row q-chunks chosen so every core runs an identical program:
  core (b,0): chunk A = rows [0:512]     (program kv extent 1024)
              chunk B = rows [1536:2048] (program kv extent 2048)
  core (b,1): chunk A = rows [512:1024]  (kv extent 1024)
              chunk B = rows [1024:1536] (kv extent 2048, data extent 1536)
Causality inside the rectangles is enforced with per-core {0,1} multiplicative
masks supplied as data, so the instruction stream is core-independent (SPMD).

Everything on-device lives transposed ([feature, token]): softmax denominators
come out of the TensorEngine via a ones-column appended to V, and no on-device
transposes are needed; the host transposes x in and the output out.

Inputs/weights/activations are bf16 (PE at full rate, fp32 PSUM accumulation);
the l/normalization path is fp32. Score matmuls for a head pair run on PE
row-groups 0-63 / 64-127 concurrently (contract dim is 64).

The emission order interleaves PE-heavy projection work into the ACT-bound
attention phases: K/V projections for kv [1024:2048] and the chunk-B Q
projection are spread between chunk-A head pairs; the chunk-A output
projection is spread between chunk-B head pairs.
"""

import numpy as np
import ml_dtypes

B, T, C, NH, D = 4, 2048, 1024, 16, 64
P = 128
CH = 512                # q-chunk size
KV_EXT = (1024, 2048)   # program kv extent for chunk A / chunk B

_CACHE = {}


def _build():
    import concourse.bacc as bacc
    import concourse.tile as tile
    import concourse.mybir as mybir
    from concourse.bass import ts, ds

    f32 = mybir.dt.float32
    bf16 = mybir.dt.bfloat16
    ID = mybir.ActivationFunctionType.Identity
    EXP = mybir.ActivationFunctionType.Exp
    COPY = mybir.ActivationFunctionType.Copy
    MUL = mybir.AluOpType.mult
    ADD = mybir.AluOpType.add

    nc = bacc.Bacc("TRN2", target_bir_lowering=False, debug=False, num_devices=8)

    def din(name, shape, dt=bf16):
        return nc.dram_tensor(name, list(shape), dt, kind="ExternalInput").ap()

    xqT = din("xqT", (C, 2 * CH))    # x^T, this core's q rows (A then B)
    xkvT = din("xkvT", (C, T))       # x^T, full batch (for K/V)
    wqT = din("wqT", (C, C))         # (Wq/8)^T
    wkT = din("wkT", (C, C))
    wvT = din("wvT", (C, C))
    woT = din("woT", (C, C))
    bq = din("bq", (P, C // P), f32)     # bq/8, chunked [128, 8]
    bk = din("bk", (P, C // P), f32)
    bo = din("bo", (P, C // P), f32)
    maskA = din("maskA", (KV_EXT[0], CH))     # {0,1}, [kv, q] chunk A
    maskB = din("maskB", (1024, CH))          # chunk B, kv in [1024:2048]
    out = nc.dram_tensor("out", [C, 2 * CH], f32, kind="ExternalOutput").ap()

    KC = C // P        # 8 contraction chunks
    NT = T // P        # 16 kv chunks of the full batch

    wq_v = wqT.rearrange("(ko p) m -> p ko m", p=P)
    wk_v = wkT.rearrange("(ko p) m -> p ko m", p=P)
    wo_v = woT.rearrange("(ko p) m -> p ko m", p=P)
    wv_v = wvT.rearrange("(ko p) c -> p ko c", p=P)
    xkv_v = xkvT.rearrange("(ko p) t -> p ko t", p=P)
    xq_v = xqT.rearrange("(ko p) t -> p ko t", p=P)
    maskA_v = maskA.rearrange("(ko p) q -> p ko q", p=P)
    maskB_v = maskB.rearrange("(ko p) q -> p ko q", p=P)

    from contextlib import ExitStack
    with ExitStack() as ctx:
        tc = ctx.enter_context(tile.TileContext(nc))

        consts = ctx.enter_context(tc.tile_pool(name="consts", bufs=1))
        big = ctx.enter_context(tc.tile_pool(name="big", bufs=1))
        wpool = ctx.enter_context(tc.tile_pool(name="w", bufs=2))
        xkpool = ctx.enter_context(tc.tile_pool(name="xk", bufs=2))
        xvpool = ctx.enter_context(tc.tile_pool(name="xv", bufs=2))
        qpool = ctx.enter_context(tc.tile_pool(name="q", bufs=1))
        mpool = ctx.enter_context(tc.tile_pool(name="m", bufs=1))
        xqpool = ctx.enter_context(tc.tile_pool(name="xq", bufs=1))
        ptpool = ctx.enter_context(tc.tile_pool(name="pt", bufs=4))
        ctxpool = ctx.enter_context(tc.tile_pool(name="ctx", bufs=1))
        lpool = ctx.enter_context(tc.tile_pool(name="l", bufs=2))
        l0pool = ctx.enter_context(tc.tile_pool(name="l0", bufs=2))
        lbpool = ctx.enter_context(tc.tile_pool(name="lb", bufs=2))
        cspool = ctx.enter_context(tc.tile_pool(name="cs", bufs=3))
        opool = ctx.enter_context(tc.tile_pool(name="o", bufs=2))
        psumP = ctx.enter_context(tc.tile_pool(name="psumP", bufs=2, space="PSUM"))
        psumS = ctx.enter_context(tc.tile_pool(name="psumS", bufs=2, space="PSUM"))
        psumX = ctx.enter_context(tc.tile_pool(name="psumX", bufs=2, space="PSUM"))

        bq_sb = consts.tile([P, KC], f32)
        bk_sb = consts.tile([P, KC], f32)
        bo_sb = consts.tile([P, KC], f32)
        nc.sync.dma_start(bq_sb[:], bq)
        nc.sync.dma_start(bk_sb[:], bk)
        nc.sync.dma_start(bo_sb[:], bo)

        KT_sb = big.tile([P, KC, T], bf16)          # K^T  [c, t]
        V_sb = big.tile([P, NT, NH, D + 1], bf16)   # V + ones col per chunk/head
        nc.vector.memset(V_sb[:, :, :, D : D + 1], 1.0)
        wvt0 = big.tile([P, KC, CH], bf16)          # Wv^T halves, resident
        wvt1 = big.tile([P, KC, CH], bf16)
        nc.sync.dma_start(wvt0[:], wv_v[:, :, 0:CH])
        nc.sync.dma_start(wvt1[:], wv_v[:, :, CH:C])
        wvt = [wvt0, wvt1]

        # ---------- emission helpers ----------
        XK = {}

        def kt_proj(ft, m0, m1):
            """KT[:, m0:m1, 512*ft:...] from a cached xk tile."""
            if ft not in XK:
                XK[ft] = xkpool.tile([P, KC, 512], bf16, tag="xk",
                                     name=f"xk{ft}")
                nc.sync.dma_start(XK[ft][:], xkv_v[:, :, ds(512 * ft, 512)])
            xk = XK[ft]
            for m in range(m0, m1):
                wt = wpool.tile([P, KC, P], bf16, tag="w", name=f"wk{ft}{m}")
                nc.sync.dma_start(wt[:], wk_v[:, :, ts(m, P)])
                ps = psumP.tile([P, 512], f32, tag="psP", name=f"pk{ft}{m}")
                for k in range(KC):
                    nc.tensor.matmul(ps[:], wt[:, k, :], xk[:, k, :],
                                     start=(k == 0), stop=(k == KC - 1))
                nc.scalar.activation(KT_sb[:, m, ds(512 * ft, 512)], ps[:],
                                     ID, bias=bk_sb[:, m : m + 1])

        def v_proj(i):
            """V rows [128*i : 128*(i+1)], all channels."""
            xv = xvpool.tile([P, KC, P], bf16, tag="xv", name=f"xv{i}")
            nc.sync.dma_start(xv[:], xkv_v[:, :, ts(i, P)])
            for chh in range(2):
                ps = psumP.tile([P, 512], f32, tag="psP", name=f"pv{i}{chh}")
                for k in range(KC):
                    nc.tensor.matmul(ps[:], xv[:, k, :], wvt[chh][:, k, :],
                                     start=(k == 0), stop=(k == KC - 1))
                nc.scalar.activation(
                    V_sb[:, i, ds(8 * chh, 8), 0:D],
                    ps.rearrange("p (h d) -> p h d", d=D), COPY)

        QT = {}

        def q_proj(qc, m0, m1):
            if qc not in QT:
                QT[qc] = qpool.tile([P, KC, CH], bf16, tag=f"qt{qc}",
                                    name=f"qt{qc}")
            if ("xq", qc) not in QT:
                QT[("xq", qc)] = xqpool.tile([P, KC, CH], bf16, tag="xq",
                                             name=f"xq{qc}")
                nc.sync.dma_start(QT[("xq", qc)][:],
                                  xq_v[:, :, ds(CH * qc, CH)])
            xq = QT[("xq", qc)]
            for m in range(m0, m1):
                wt = wpool.tile([P, KC, P], bf16, tag="w", name=f"wq{qc}{m}")
                nc.sync.dma_start(wt[:], wq_v[:, :, ts(m, P)])
                ps = psumP.tile([P, CH], f32, tag="psP", name=f"pq{qc}{m}")
                for k in range(KC):
                    nc.tensor.matmul(ps[:], wt[:, k, :], xq[:, k, :],
                                     start=(k == 0), stop=(k == KC - 1))
                nc.scalar.activation(QT[qc][:, m, :], ps[:], ID,
                                     bias=bq_sb[:, m : m + 1])

        def o_proj(qc, ctxT, m):
            wt = wpool.tile([P, KC, P], bf16, tag="w", name=f"wo{qc}{m}")
            nc.sync.dma_start(wt[:], wo_v[:, :, ts(m, P)])
            ps = psumP.tile([P, CH], f32, tag="psP", name=f"po{qc}{m}")
            for k in range(KC):
                nc.tensor.matmul(ps[:], wt[:, k, :], ctxT[:, k, :],
                                 start=(k == 0), stop=(k == KC - 1))
            o_sb = opool.tile([P, CH], f32, tag="o", name=f"o{qc}{m}")
            nc.scalar.activation(o_sb[:], ps[:], ID, bias=bo_sb[:, m : m + 1])
            nc.sync.dma_start(out[ts(m, P), ds(CH * qc, CH)], o_sb[:])

        def attn_pair(qc, hp, msk, ctxT):
            E = KV_EXT[qc]
            NKV = E // P
            ctx_ps = [psumX.tile([P, CH], f32, tag="psX", name=f"psX{qc}{hp}{i}")
                      for i in range(2)]
            for kvc in range(NKV):
                st = psumS.tile([P, 2, CH], f32, tag="psS",
                                name=f"psS{qc}{hp}{kvc}")
                for hh in range(2):
                    # contract dim 64 at PE row-group 64*hh: the two heads'
                    # score matmuls run concurrently in the array
                    nc.tensor.matmul(
                        st[:, hh, :],
                        KT_sb[ds(64 * hh, 64), hp, ds(P * kvc, P)],
                        QT[qc][ds(64 * hh, 64), hp, :],
                        start=True, stop=True)
                pt = ptpool.tile([P, 2, CH], bf16, tag="pt",
                                 name=f"pt{qc}{hp}{kvc}")
                nc.scalar.activation(pt[:], st[:], EXP)
                mi = kvc if qc == 0 else kvc - NKV // 2
                if mi >= 0:   # causal mask (chunk A: all; chunk B: kv >= 1024)
                    nc.vector.tensor_tensor(
                        pt[:], pt[:],
                        msk[:, mi : mi + 1, :].to_broadcast((P, 2, CH)), MUL)
                for hh in range(2):
                    nc.tensor.matmul(
                        ctx_ps[hh][0 : D + 1, :],
                        V_sb[:, kvc, 2 * hp + hh, :],
                        pt[:, hh, :],
                        start=(kvc == 0), stop=(kvc == NKV - 1))
            # Epilogue. Free the PSUM banks fast (reciprocal of row D + DVE
            # copy of rows [0:D) to SBUF); the 1/l row is hopped to physical
            # partition 0 (the only one HW partition_broadcast reads) on the
            # GpSimd DMA queue, broadcast on GpSimd, normalized on DVE, and
            # partition-remapped into ctxT with a GpSimd-queued DMA.
            cs = []
            for hh in range(2):
                l_sb = lpool.tile([P, CH], f32, tag="l", name=f"l{qc}{hp}{hh}")
                nc.vector.reciprocal(l_sb[D : D + 1, :],
                                     ctx_ps[hh][D : D + 1, :])
                l0 = l0pool.tile([1, CH], f32, tag="l0", name=f"l0{qc}{hp}{hh}")
                nc.gpsimd.dma_start(l0[:], l_sb[D : D + 1, :])
                c_scr = cspool.tile([P, CH], f32, tag="cs",
                                    name=f"cs{qc}{hp}{hh}")
                nc.vector.tensor_copy(c_scr[0:D, :], ctx_ps[hh][0:D, :])
                cs.append((l0, c_scr))
            for hh in range(2):
                l0, c_scr = cs[hh]
                linv = lbpool.tile([P, CH], f32, tag="lb", name=f"lb{qc}{hp}{hh}")
                nc.gpsimd.partition_broadcast(linv[0:D, :], l0[:], channels=D)
                if hh == 0:
                    nc.vector.tensor_tensor(ctxT[0:D, hp, :], c_scr[0:D, :],
                                            linv[0:D, :], MUL)
                else:
                    c2 = cspool.tile([P, CH], bf16, tag="cs2",
                                     name=f"cs2{qc}{hp}")
                    nc.vector.tensor_tensor(c2[0:D, :], c_scr[0:D, :],
                                            linv[0:D, :], MUL)
                    nc.gpsimd.dma_start(ctxT[ds(64, 64), hp, :], c2[0:D, :])

        # ---------- emission schedule ----------
        # prologue: K/V for kv [0:1024], Q for chunk A
        for ft in range(2):
            kt_proj(ft, 0, 4); kt_proj(ft, 4, 8)
        for i in range(8):
            v_proj(i)
        q_proj(0, 0, 4); q_proj(0, 4, 8)

        mskA = mpool.tile([P, KC, CH], bf16, tag="mask", name="mA")
        nc.sync.dma_start(mskA[:], maskA_v)

        # chunk A attention, with kv[1024:2048] K/V projections and the
        # chunk-B Q projection interleaved as PE filler
        ctxT_A = ctxpool.tile([P, KC, CH], bf16, tag="ctxA", name="ctxA")
        fillers = ([lambda ft=ft, m0=m0: kt_proj(ft, m0, m0 + 4)
                    for ft in (2, 3) for m0 in (0, 4)]
                   + [lambda i=i: v_proj(i) for i in range(8, 16)]
                   + [lambda m0=m0: q_proj(1, m0, m0 + 4) for m0 in (0, 4)])
        fi = 0
        for hp in range(NH // 2):
            attn_pair(0, hp, mskA, ctxT_A)
            take = (len(fillers) - fi + (NH // 2 - hp) - 1) // (NH // 2 - hp)
            for _ in range(take):
                if fi < len(fillers):
                    fillers[fi](); fi += 1
        while fi < len(fillers):
            fillers[fi](); fi += 1

        # chunk B attention, with chunk-A output projection interleaved
        mskB = mpool.tile([P, KC, CH], bf16, tag="mask", name="mB")
        nc.sync.dma_start(mskB[:], maskB_v)
        ctxT_B = ctxpool.tile([P, KC, CH], bf16, tag="ctxB", name="ctxB")
        for hp in range(NH // 2):
            attn_pair(1, hp, mskB, ctxT_B)
            o_proj(0, ctxT_A, hp)
        for m in range(NH // 2, KC):
            o_proj(0, ctxT_A, m)
        for m in range(KC):
            o_proj(1, ctxT_B, m)

    nc.compile()
    return nc


def _shard_inputs(x, Wq, bq, bk_, bv, bo, WqT, WkT, WvT, WoT):
    """Build the 8 per-core input maps (bf16 data tensors, fp32 biases).

    bv is folded into the output-projection bias: ctx = ctx0 + 1*bv^T, so
    out = ctx0 @ Wo^T + (bo + Wo @ bv)."""
    bf = ml_dtypes.bfloat16
    in_maps = []
    rows = {0: (np.arange(0, 512), np.arange(1536, 2048)),
            1: (np.arange(512, 1024), np.arange(1024, 1536))}
    kv = np.arange(T)
    bq8 = np.ascontiguousarray((bq / 8.0).reshape(C // P, P).T)
    bk8 = np.ascontiguousarray(bk_.reshape(C // P, P).T)
    bo_f = bo + WoT.T @ bv
    bo8 = np.ascontiguousarray(bo_f.reshape(C // P, P).T)
    wq16, wk16 = WqT.astype(bf), WkT.astype(bf)
    wv16, wo16 = WvT.astype(bf), WoT.astype(bf)
    for b in range(B):
        xT = np.ascontiguousarray(x[b].T).astype(bf)     # (C, T)
        for h in range(2):
            qA, qB = rows[h]
            xqT = np.ascontiguousarray(xT[:, np.concatenate([qA, qB])])
            mA = (kv[:1024, None] <= qA[None, :]).astype(bf)
            mB = (kv[1024:, None] <= qB[None, :]).astype(bf)
            in_maps.append({
                "xqT": xqT, "xkvT": xT,
                "wqT": wq16, "wkT": wk16, "wvT": wv16, "woT": wo16,
                "bq": bq8, "bk": bk8, "bo": bo8,
                "maskA": np.ascontiguousarray(mA),
                "maskB": np.ascontiguousarray(mB),
            })
    return in_maps


def kernel(x, Wq, bq, Wk, bk, Wv, bv, Wo, bo):
    from concourse.bass_utils import run_bass_kernel_spmd

    x = np.asarray(x, np.float32)
    Wq = np.asarray(Wq, np.float32); bq = np.asarray(bq, np.float32)
    Wk = np.asarray(Wk, np.float32); bk = np.asarray(bk, np.float32)
    Wv = np.asarray(Wv, np.float32); bv = np.asarray(bv, np.float32)
    Wo = np.asarray(Wo, np.float32); bo = np.asarray(bo, np.float32)

    if "nc" not in _CACHE:
        _CACHE["nc"] = _build()
    nc = _CACHE["nc"]

    WqT = np.ascontiguousarray(Wq.T / 8.0)
    WkT = np.ascontiguousarray(Wk.T)
    WvT = np.ascontiguousarray(Wv.T)
    WoT = np.ascontiguousarray(Wo.T)
    in_maps = _shard_inputs(x, Wq, bq, bk, bv, bo, WqT, WkT, WvT, WoT)

    res = run_bass_kernel_spmd(nc, in_maps, core_ids=list(range(8)))
    outf = np.empty((B, T, C), np.float32)
    rows = {0: (np.arange(0, 512), np.arange(1536, 2048)),
            1: (np.arange(512, 1024), np.arange(1024, 1536))}
    for b in range(B):
        for h in range(2):
            o = res.results[2 * b + h]["out"]          # (C, 1024) transposed
            qA, qB = rows[h]
            outf[b, qA, :] = o[:, :512].T
            outf[b, qB, :] = o[:, 512:].T
    return outf



# revision 2
# speedup vs baseline: 1.0174x; 1.0174x over previous
"""Causal multi-head attention (B=4, T=2048, C=1024, 16 heads) on 8 TRN2 cores.

Sharding v2: core (b, h) = (batch b, head-half h).  Each core projects
Q/K/V for its 8 heads only (no cross-core K/V redundancy), runs causally
tiled attention (q tiles of 512, kv extent (qc+1)*512 -- no fully-masked
tiles are ever computed), computes the partial output projection over its
512 ctx features, and the host sums the two partials per batch during the
unshard (out = p0 + p1 + bo + Wo@bv; bv is folded out via softmax rows
summing to 1, so the device never needs any V/O bias).

On-device layout is transposed ([feature, token]) like v1; softmax
denominators come from a ones-column appended to V.  Fixes vs v1:
  * reciprocal: [2,512] reciprocal_approx_fast per pair (was 32x 3.3us
    single-lane [1,512] full reciprocals = 106us DVE),
  * 1/l partition-broadcast via a tiny selector matmul on the PE instead
    of GpSimd partition_broadcast,
  * projection bias epilogues on DVE tensor_scalar (ACT does only exp),
  * all weights resident in SBUF (no per-matmul weight DMAs),
  * software-pipelined emission: scores(j+1) and 2-matmul projection
    filler chunks are emitted *before* ctx(j) so the PE never stalls
    behind the exp->mask chain, and the pair epilogue's PE work is
    deferred into the next pair to avoid pipeline bubbles.
"""

import numpy as np
import ml_dtypes

B, T, C, NH, D = 4, 2048, 1024, 16, 64
P = 128
HC = 512            # channels per head-half (8 heads x 64)
QT_ = 512           # q tile size
NQC = T // QT_      # 4 q tiles
NKV = T // P        # 16 kv blocks

_CACHE = {}


def _build():
    import concourse.bacc as bacc
    import concourse.tile as tile
    import concourse.mybir as mybir
    from concourse.bass import ts, ds

    f32 = mybir.dt.float32
    bf16 = mybir.dt.bfloat16
    EXP = mybir.ActivationFunctionType.Exp
    MUL = mybir.AluOpType.mult

    nc = bacc.Bacc("TRN2", target_bir_lowering=False, debug=False, num_devices=8)

    def din(name, shape, dt=bf16):
        return nc.dram_tensor(name, list(shape), dt, kind="ExternalInput").ap()

    # All inputs come pre-arranged on the host so every DMA slice is
    # contiguous per partition (strided (k p)->p k views halve the ring
    # throughput, which gated the prologue).
    x_v = din("xT", (P, 4, 8, QT_))   # x[b]^T as [p, tt, k, u]
    wq_v = din("wqT", (P, 4, 8, P))   # (Wq^T/8) cols_h as [p, ko, k, u]
    wk_v = din("wkT", (P, 4, 8, P))
    wv_v = din("wvT", (P, 8, HC))     # moving operand, loaded whole
    wo_v = din("woT", (P, 4, C))      # Wo^T rows of this half
    bqk = din("bqk", (P, 8), f32)     # cols 0:4 bq/8 chunks, 4:8 bk chunks
    mask_v = din("mask", (P, 4, QT_))  # tril block pattern as [p, jl, q]
    sel = din("sel", (2, 2, D))       # selector for 1/l broadcast matmul
    out = nc.dram_tensor("out", [C, T], bf16, kind="ExternalOutput").ap()

    from contextlib import ExitStack
    with ExitStack() as ctx:
        tc = ctx.enter_context(tile.TileContext(nc))

        consts = ctx.enter_context(tc.tile_pool(name="consts", bufs=1))
        big = ctx.enter_context(tc.tile_pool(name="big", bufs=1))
        ptpool = ctx.enter_context(tc.tile_pool(name="pt", bufs=4))
        cspool = ctx.enter_context(tc.tile_pool(name="cs", bufs=3))
        lpool = ctx.enter_context(tc.tile_pool(name="l", bufs=2))
        lipool = ctx.enter_context(tc.tile_pool(name="li", bufs=2))
        lbpool = ctx.enter_context(tc.tile_pool(name="lb", bufs=2))
        sbpool = ctx.enter_context(tc.tile_pool(name="sb", bufs=2))
        opool = ctx.enter_context(tc.tile_pool(name="o", bufs=3))
        psumP = ctx.enter_context(tc.tile_pool(name="psumP", bufs=2, space="PSUM"))
        psumS = ctx.enter_context(tc.tile_pool(name="psumS", bufs=2, space="PSUM"))
        psumX = ctx.enter_context(tc.tile_pool(name="psumX", bufs=1, space="PSUM"))

        # ---- resident tiles ----
        bqk_sb = consts.tile([P, 8], f32)
        sel_sb = consts.tile([2, 2, D], bf16)
        mask_sb = consts.tile([P, 4, QT_], bf16)
        xT_sb = big.tile([P, 4, 8, QT_], bf16)
        wq_sb = big.tile([P, 4, 8, P], bf16)
        wk_sb = big.tile([P, 4, 8, P], bf16)
        wv_sb = big.tile([P, 8, HC], bf16)
        wo_sb = big.tile([P, 4, C], bf16)
        KT_sb = big.tile([P, 4, T], bf16)
        QT_sb = big.tile([P, 4, T], bf16)
        V_sb = big.tile([P, NKV, 8, D + 1], bf16)
        ctxT_sb = big.tile([P, 4, T], bf16)

        # Input DMAs, deadline-sorted across three rings.  The sync ring
        # starts transfers ~4us before the scalar/gpsimd rings, so the
        # earliest-needed tensors (wk/wq ko=0, x tt=0, bqk, mask) go there.
        nc.sync.dma_start(wk_sb[:, 0, :, :], wk_v[:, 0, :, :])
        nc.sync.dma_start(xT_sb[:, 0, 0:4, :], x_v[:, 0, 0:4, :])
        nc.sync.dma_start(wq_sb[:, 0, :, :], wq_v[:, 0, :, :])
        nc.sync.dma_start(bqk_sb[:], bqk)
        nc.sync.dma_start(mask_sb[:], mask_v)
        nc.sync.dma_start(sel_sb[:], sel)
        for tt in range(1, 4):
            nc.sync.dma_start(xT_sb[:, tt, :, :], x_v[:, tt, :, :])
        nc.scalar.dma_start(wv_sb[:], wv_v)
        nc.scalar.dma_start(wo_sb[:], wo_v)
        nc.gpsimd.dma_start(xT_sb[:, 0, 4:8, :], x_v[:, 0, 4:8, :])
        for ko in range(1, 4):
            nc.gpsimd.dma_start(wk_sb[:, ko, :, :], wk_v[:, ko, :, :])
            nc.gpsimd.dma_start(wq_sb[:, ko, :, :], wq_v[:, ko, :, :])

        nc.vector.memset(V_sb[:, :, :, D : D + 1], 1.0)
        # preload the exp table set during the DMA prologue
        dscr = consts.tile([1, 8], f32)
        dout = consts.tile([1, 8], f32)
        nc.vector.memset(dscr[:], 0.0)
        nc.scalar.activation(dout[:], dscr[:], EXP)
        # HAM warm-up: ~4us of dummy matmuls while the input DMAs land, so
        # the PE clock is at 2.4GHz (not the cold 1.2) when real work starts.
        wscr = consts.tile([P, QT_], bf16)
        nc.vector.memset(wscr[:], 0.0)
        for w in range(30):
            wps = psumS.tile([P, 2, QT_], f32, tag="st", name=f"w{w}")
            nc.tensor.matmul(wps[:, 0, :], wscr[:, 0:P], wscr[:],
                             start=True, stop=True)

        # ---------- emission units (generators yield every ~2 matmuls) ----
        emitted = set()

        def kq_gen(i, tt):
            """K and Q projections for pair i, token chunk tt (16 MMs)."""
            for (w_sb, dst, bcol) in ((wk_sb, KT_sb, 4 + i), (wq_sb, QT_sb, i)):
                ps = psumP.tile([P, QT_], f32, tag="proj",
                                name=f"pp{bcol}{i}{tt}")
                for k in range(8):
                    nc.tensor.matmul(ps[:], w_sb[:, i, k, :],
                                     xT_sb[:, tt, k, :],
                                     start=(k == 0), stop=(k == 7))
                    if k % 2 == 1 and k < 7:
                        yield
                nc.vector.tensor_scalar_add(dst[:, i, ds(QT_ * tt, QT_)],
                                            ps[:], bqk_sb[:, bcol : bcol + 1])
                yield
            emitted.add(("kq", i, tt))

        def v_gen(j):
            """V projection for kv block j, all 8 heads (8 MMs)."""
            ps = psumP.tile([P, HC], f32, tag="proj", name=f"pv{j}")
            for k in range(8):
                nc.tensor.matmul(ps[:],
                                 xT_sb[:, j // 4, k, ds(P * (j % 4), P)],
                                 wv_sb[:, k, :],
                                 start=(k == 0), stop=(k == 7))
                if k % 2 == 1 and k < 7:
                    yield
            nc.vector.tensor_copy(V_sb[:, j, :, 0:D],
                                  ps.rearrange("p (h d) -> p h d", d=D))
            emitted.add(("v", j))
            yield

        def o_gen(ko, tt):
            """Partial output projection rows 128ko, token chunk tt (4 MMs)."""
            ps = psumP.tile([P, QT_], f32, tag="proj", name=f"po{ko}{tt}")
            for k in range(4):
                nc.tensor.matmul(ps[:], wo_sb[:, k, ts(ko, P)],
                                 ctxT_sb[:, k, ds(QT_ * tt, QT_)],
                                 start=(k == 0), stop=(k == 3))
                if k == 1:
                    yield
            o_sb = opool.tile([P, QT_], bf16, tag="o", name=f"o{ko}{tt}")
            nc.vector.tensor_copy(o_sb[:], ps[:])
            eng = (nc.sync, nc.gpsimd)[(ko + 8 * tt) % 2]
            eng.dma_start(out[ts(ko, P), ds(QT_ * tt, QT_)], o_sb[:])
            yield

        # ---------- filler scheduler ----------
        FQ = []
        for i in (1, 2, 3):
            FQ.append(("kq", i, 0))
        FQ += [("v", 4), ("v", 5)]
        for i in range(4):
            FQ.append(("kq", i, 1))
        FQ += [("v", 6), ("v", 7)]
        for i in range(4):
            FQ.append(("kq", i, 2))
        FQ += [("v", 8), ("v", 9), ("v", 10), ("v", 11)]
        for i in range(4):
            FQ.append(("kq", i, 3))
        FQ += [("v", 12), ("v", 13), ("v", 14), ("v", 15)]
        FQ += [("o", ko, 0) for ko in range(8)]
        FQ += [("o", ko, 1) for ko in range(8)]
        FQ += [("o", ko, 2) for ko in range(8)]

        def make_gen(key):
            if key[0] == "kq":
                return kq_gen(key[1], key[2])
            if key[0] == "v":
                return v_gen(key[1])
            return o_gen(key[1], key[2])

        state = {"cur": None, "curkey": None}
        ctx_done = set()  # q-chunks whose ctxT is fully written (stage B out)

        def next_key():
            for idx, key in enumerate(FQ):
                if key[0] == "o" and key[2] not in ctx_done:
                    continue  # ctxT for that chunk not complete yet
                FQ.pop(idx)
                return key
            return None

        def pump(steps):
            while steps > 0:
                if state["cur"] is None:
                    key = next_key()
                    if key is None:
                        return
                    state["cur"] = make_gen(key)
                    state["curkey"] = key
                try:
                    next(state["cur"])
                    steps -= 1
                except StopIteration:
                    state["cur"] = None
                    state["curkey"] = None

        def drain():
            if state["cur"] is not None:
                for _ in state["cur"]:
                    pass
                state["cur"] = None
                state["curkey"] = None

        def force(key):
            if key in emitted:
                return
            drain()
            if key in emitted:
                return  # the drained in-flight unit was this key
            if key in FQ:
                FQ.remove(key)
            for _ in make_gen(key):
                pass

        # ---------- attention ----------
        def scores(qc, i, j):
            st = psumS.tile([P, 2, QT_], f32, tag="st", name=f"st{qc}{i}{j}")
            for hh in range(2):
                nc.tensor.matmul(
                    st[:, hh, :],
                    KT_sb[ds(D * hh, D), i, ts(j, P)],
                    QT_sb[ds(D * hh, D), i, ds(QT_ * qc, QT_)],
                    start=True, stop=True)
            return st

        def exp_mask(qc, i, j, st):
            pt = ptpool.tile([P, 2, QT_], bf16, tag="pt", name=f"pt{qc}{i}{j}")
            nc.scalar.activation(pt[:], st[:], EXP)
            jl = j - 4 * qc
            if jl >= 0:
                nc.vector.tensor_tensor(
                    pt[:], pt[:],
                    mask_sb[:, jl : jl + 1, :].to_broadcast((P, 2, QT_)), MUL)
            return pt

        def ctx_mm(i, j, jmax, ctx_ps, pt):
            for hh in range(2):
                nc.tensor.matmul(
                    ctx_ps[0 : D + 1, hh, :],
                    V_sb[:, j, 2 * i + hh, :],
                    pt[:, hh, :],
                    start=(j == 0), stop=(j == jmax))

        deferred = []

        def epilogue_a(qc, i, ctx_ps):
            """Drain the pair's PSUM, stage l rows, 1/l; defer the PE/DVE
            normalization (stage B) so its wait doesn't bubble the PE."""
            cs = cspool.tile([D + 1, 2, QT_], f32, tag="cs", name=f"cs{qc}{i}")
            nc.vector.tensor_copy(cs[:], ctx_ps[0 : D + 1, :, :])
            l2 = lpool.tile([2, QT_], f32, tag="l", name=f"l{qc}{i}")
            nc.gpsimd.dma_start(l2[:], cs[D : D + 1, :, :])
            li = lipool.tile([2, QT_], f32, tag="li", name=f"li{qc}{i}")
            nc.vector.reciprocal_approx_fast(li[:], l2[:])
            lb = lbpool.tile([2, QT_], bf16, tag="lb", name=f"lb{qc}{i}")
            nc.vector.tensor_copy(lb[:], li[:])
            deferred.append((qc, i, cs, lb))

        def epilogue_b():
            while deferred:
                qc, i, cs, lb = deferred.pop(0)
                bct = psumS.tile([P, 2, QT_], f32, tag="st", name=f"bc{qc}{i}")
                for hh in range(2):
                    bc = bct[0:D, hh, :]
                    nc.tensor.matmul(bc, sel_sb[0:2, hh, :], lb[:],
                                     start=True, stop=True)
                    if hh == 0:
                        nc.vector.tensor_tensor(
                            ctxT_sb[0:D, i, ds(QT_ * qc, QT_)],
                            cs[0:D, 0, :], bc, MUL)
                    else:
                        sb = sbpool.tile([D, QT_], bf16, tag="sb",
                                         name=f"sb{qc}{i}")
                        nc.vector.tensor_tensor(sb[:], cs[0:D, 1, :], bc, MUL)
                        nc.gpsimd.dma_start(
                            ctxT_sb[ds(D, D), i, ds(QT_ * qc, QT_)], sb[:])
                if i == 3:
                    ctx_done.add(qc)

        # ---------- emission schedule ----------
        force(("kq", 0, 0))
        for j in range(4):
            force(("v", j))

        # Filler pacing: steps each qc's groups must emit so the next qc's
        # K/Q/V are projected before it starts (else forced PE bursts starve
        # ACT at qc boundaries).
        RATE = {0: 5.0, 1: 2.7, 2: 2.8, 3: 2.2}
        pairs = [(qc, i) for qc in range(NQC) for i in range(4)]
        pump_acc = 0.0
        carried = None
        carried_key = None
        for idx, (qc, i) in enumerate(pairs):
            jmax = (qc + 1) * 4 - 1
            force(("kq", i, qc))
            for j in range(jmax + 1):
                if ("v", j) not in emitted:
                    force(("v", j))
            ctx_ps = psumX.tile([P, 2, QT_], f32, tag="ctx",
                                name=f"ctx{qc}{i}")
            # kv blocks in groups of 2: one [sc,sc] burst per group keeps
            # row-grouped score MMs contiguous (each boundary between the
            # 64-contract score pairs and full-array MMs costs ~105ns of
            # exposed LDWEIGHTS).
            if carried_key == (qc, i):
                stg = carried
            else:
                stg = [scores(qc, i, 0), scores(qc, i, 1)]
            carried = None
            carried_key = None
            for g in range(0, jmax + 1, 2):
                if g == 2:
                    epilogue_b()
                nxt = ([scores(qc, i, g + 2), scores(qc, i, g + 3)]
                       if g + 2 <= jmax else None)
                pt0 = exp_mask(qc, i, g, stg[0])
                pt1 = exp_mask(qc, i, g + 1, stg[1])
                pump_acc += RATE[qc]
                take = int(pump_acc)
                pump_acc -= take
                pump(take)
                if g == jmax - 1 and idx + 1 < len(pairs):
                    nqc, ni = pairs[idx + 1]
                    if ("kq", ni, nqc) in emitted:
                        # pre-emit the next pair's first score group so ACT
                        # never idles across the pair boundary
                        carried = [scores(nqc, ni, 0), scores(nqc, ni, 1)]
                        carried_key = (nqc, ni)
                ctx_mm(i, g, jmax, ctx_ps, pt0)
                ctx_mm(i, g + 1, jmax, ctx_ps, pt1)
                stg = nxt
            epilogue_a(qc, i, ctx_ps)
        drain()
        # leftover fillers keep the PE busy while the last pair's 1/l chain
        # (cs copy -> hop -> recip -> cast) completes
        while FQ:
            key = FQ.pop(0)
            for _ in make_gen(key):
                pass
        epilogue_b()
        for ko in range(8):
            for _ in o_gen(ko, 3):
                pass

    nc.compile()
    return nc


def _shard_inputs(x, Wq, bq, Wk, bk, Wv, Wo):
    bf = ml_dtypes.bfloat16

    def kchunks(wT_cols):  # (1024, 512) -> [p, ko, k, u] (128, 4, 8, 128)
        return np.ascontiguousarray(
            wT_cols.reshape(8, P, 4, P).transpose(1, 2, 0, 3))

    wqT = (Wq.T / 8.0).astype(bf)
    wkT = Wk.T.astype(bf)
    wvT = Wv.T.astype(bf)
    woT = Wo.T.astype(bf)
    kv = np.arange(QT_)
    mask_np = np.ascontiguousarray(
        (kv[:, None] <= kv[None, :]).astype(bf)
        .reshape(4, P, QT_).transpose(1, 0, 2))
    sel_np = np.zeros((2, 2, D), dtype=bf)
    sel_np[0, 0, :] = 1.0
    sel_np[1, 1, :] = 1.0
    in_maps = []
    for b in range(B):
        xT = x[b].T.astype(bf)  # (1024, 2048)
        # [128k+p, 512tt+u] -> [p, tt, k, u]
        xp = np.ascontiguousarray(
            xT.reshape(8, P, 4, QT_).transpose(1, 2, 0, 3))
        for h in range(2):
            cols = slice(HC * h, HC * (h + 1))
            bq_h = (bq[cols] / 8.0).reshape(4, P).T
            bk_h = bk[cols].reshape(4, P).T
            bqk_np = np.ascontiguousarray(
                np.concatenate([bq_h, bk_h], axis=1)).astype(np.float32)
            in_maps.append({
                "xT": xp,
                "wqT": kchunks(wqT[:, cols]),
                "wkT": kchunks(wkT[:, cols]),
                "wvT": np.ascontiguousarray(
                    wvT[:, cols].reshape(8, P, HC)[:, :, :]
                    .transpose(1, 0, 2)),
                "woT": np.ascontiguousarray(
                    woT[cols, :].reshape(4, P, C).transpose(1, 0, 2)),
                "bqk": bqk_np,
                "mask": mask_np,
                "sel": sel_np,
            })
    return in_maps


def kernel(x, Wq, bq, Wk, bk, Wv, bv, Wo, bo):
    from concourse.bass_utils import run_bass_kernel_spmd

    x = np.asarray(x, np.float32)
    Wq = np.asarray(Wq, np.float32); bq = np.asarray(bq, np.float32)
    Wk = np.asarray(Wk, np.float32); bk = np.asarray(bk, np.float32)
    Wv = np.asarray(Wv, np.float32); bv = np.asarray(bv, np.float32)
    Wo = np.asarray(Wo, np.float32); bo = np.asarray(bo, np.float32)

    if "nc" not in _CACHE:
        _CACHE["nc"] = _build()
    nc = _CACHE["nc"]

    in_maps = _shard_inputs(x, Wq, bq, Wk, bk, Wv, Wo)
    res = run_bass_kernel_spmd(nc, in_maps, core_ids=list(range(8)))

    bias = (bo + Wo @ bv).astype(np.float32)
    outf = np.empty((B, T, C), np.float32)
    for b in range(B):
        p0 = np.asarray(res.results[2 * b]["out"], dtype=np.float32)
        p1 = np.asarray(res.results[2 * b + 1]["out"], dtype=np.float32)
        outf[b] = (p0 + p1).T + bias[None, :]
    return outf


# revision 3
# speedup vs baseline: 1.0229x; 1.0055x over previous
"""Causal multi-head attention (B=4, T=2048, C=1024, 16 heads) on 8 TRN2 cores.

Sharding v2: core (b, h) = (batch b, head-half h).  Each core projects
Q/K/V for its 8 heads only (no cross-core K/V redundancy), runs causally
tiled attention (q tiles of 512, kv extent (qc+1)*512 -- no fully-masked
tiles are ever computed), computes the partial output projection over its
512 ctx features, and the host sums the two partials per batch during the
unshard (out = p0 + p1 + bo + Wo@bv; bv is folded out via softmax rows
summing to 1, so the device never needs any V/O bias).

On-device layout is transposed ([feature, token]) like v1; softmax
denominators come from a ones-column appended to V.  Fixes vs v1:
  * reciprocal: [2,512] reciprocal_approx_fast per pair (was 32x 3.3us
    single-lane [1,512] full reciprocals = 106us DVE),
  * 1/l partition-broadcast via a tiny selector matmul on the PE instead
    of GpSimd partition_broadcast,
  * projection bias epilogues on DVE tensor_scalar (ACT does only exp),
  * all weights resident in SBUF (no per-matmul weight DMAs),
  * software-pipelined emission: scores(j+1) and 2-matmul projection
    filler chunks are emitted *before* ctx(j) so the PE never stalls
    behind the exp->mask chain, and the pair epilogue's PE work is
    deferred into the next pair to avoid pipeline bubbles.
"""

import numpy as np
import ml_dtypes

B, T, C, NH, D = 4, 2048, 1024, 16, 64
P = 128
HC = 512            # channels per head-half (8 heads x 64)
QT_ = 512           # q tile size
NQC = T // QT_      # 4 q tiles
NKV = T // P        # 16 kv blocks

_CACHE = {}


def _build():
    import concourse.bacc as bacc
    import concourse.tile as tile
    import concourse.mybir as mybir
    from concourse.bass import ts, ds

    f32 = mybir.dt.float32
    bf16 = mybir.dt.bfloat16
    EXP = mybir.ActivationFunctionType.Exp
    MUL = mybir.AluOpType.mult

    nc = bacc.Bacc("TRN2", target_bir_lowering=False, debug=False, num_devices=8)

    def din(name, shape, dt=bf16):
        return nc.dram_tensor(name, list(shape), dt, kind="ExternalInput").ap()

    # All inputs come pre-arranged on the host so every DMA slice is
    # contiguous per partition (strided (k p)->p k views halve the ring
    # throughput, which gated the prologue).
    x_v = din("xT", (P, 4, 8, QT_))   # x[b]^T as [p, tt, k, u]
    wq_v = din("wqT", (P, 4, 8, P))   # (Wq^T/8) cols_h as [p, ko, k, u]
    wk_v = din("wkT", (P, 4, 8, P))
    wv_v = din("wvT", (P, 8, HC))     # moving operand, loaded whole
    wo_v = din("woT", (P, 4, C))      # Wo^T rows of this half
    bqk = din("bqk", (P, 8), f32)     # cols 0:4 bq/8 chunks, 4:8 bk chunks
    mask_v = din("mask", (P, 4, QT_))  # tril block pattern as [p, jl, q]
    sel = din("sel", (2, 2, D))       # selector for 1/l broadcast matmul
    out = nc.dram_tensor("out", [C, T], bf16, kind="ExternalOutput").ap()

    from contextlib import ExitStack
    with ExitStack() as ctx:
        tc = ctx.enter_context(tile.TileContext(nc))

        consts = ctx.enter_context(tc.tile_pool(name="consts", bufs=1))
        big = ctx.enter_context(tc.tile_pool(name="big", bufs=1))
        ptpool = ctx.enter_context(tc.tile_pool(name="pt", bufs=4))
        cspool = ctx.enter_context(tc.tile_pool(name="cs", bufs=3))
        lpool = ctx.enter_context(tc.tile_pool(name="l", bufs=2))
        lipool = ctx.enter_context(tc.tile_pool(name="li", bufs=2))
        lbpool = ctx.enter_context(tc.tile_pool(name="lb", bufs=2))
        sbpool = ctx.enter_context(tc.tile_pool(name="sb", bufs=2))
        opool = ctx.enter_context(tc.tile_pool(name="o", bufs=3))
        psumP = ctx.enter_context(tc.tile_pool(name="psumP", bufs=2, space="PSUM"))
        psumS = ctx.enter_context(tc.tile_pool(name="psumS", bufs=2, space="PSUM"))
        psumX = ctx.enter_context(tc.tile_pool(name="psumX", bufs=1, space="PSUM"))

        # ---- resident tiles ----
        bqk_sb = consts.tile([P, 8], f32)
        sel_sb = consts.tile([2, 2, D], bf16)
        mask_sb = consts.tile([P, 4, QT_], bf16)
        xT_sb = big.tile([P, 4, 8, QT_], bf16)
        wq_sb = big.tile([P, 4, 8, P], bf16)
        wk_sb = big.tile([P, 4, 8, P], bf16)
        wv_sb = big.tile([P, 8, HC], bf16)
        wo_sb = big.tile([P, 4, C], bf16)
        KT_sb = big.tile([P, 4, T], bf16)
        QT_sb = big.tile([P, 4, T], bf16)
        V_sb = big.tile([P, NKV, 8, D + 1], bf16)
        ctxT_sb = big.tile([P, 4, T], bf16)

        # Input DMAs, deadline-sorted across three rings.  The sync ring
        # starts transfers ~4us before the scalar/gpsimd rings, so the
        # earliest-needed tensors (wk/wq ko=0, x tt=0, bqk, mask) go there.
        nc.sync.dma_start(wk_sb[:, 0, :, :], wk_v[:, 0, :, :])
        nc.sync.dma_start(xT_sb[:, 0, 0:4, :], x_v[:, 0, 0:4, :])
        nc.sync.dma_start(xT_sb[:, 0, 4:8, :], x_v[:, 0, 4:8, :])
        nc.sync.dma_start(wq_sb[:, 0, :, :], wq_v[:, 0, :, :])
        nc.sync.dma_start(bqk_sb[:], bqk)
        nc.sync.dma_start(mask_sb[:], mask_v)
        nc.sync.dma_start(sel_sb[:], sel)
        for tt in range(1, 4):
            nc.sync.dma_start(xT_sb[:, tt, :, :], x_v[:, tt, :, :])
        nc.scalar.dma_start(wv_sb[:], wv_v)
        nc.scalar.dma_start(wo_sb[:], wo_v)
        for ko in range(1, 4):
            nc.gpsimd.dma_start(wk_sb[:, ko, :, :], wk_v[:, ko, :, :])
            nc.gpsimd.dma_start(wq_sb[:, ko, :, :], wq_v[:, ko, :, :])

        nc.vector.memset(V_sb[:, :, :, D : D + 1], 1.0)
        # preload the exp table set during the DMA prologue
        dscr = consts.tile([1, 8], f32)
        dout = consts.tile([1, 8], f32)
        nc.vector.memset(dscr[:], 0.0)
        nc.scalar.activation(dout[:], dscr[:], EXP)
        # HAM warm-up: ~4us of dummy matmuls while the input DMAs land, so
        # the PE clock is at 2.4GHz (not the cold 1.2) when real work starts.
        wscr = consts.tile([P, QT_], bf16)
        nc.vector.memset(wscr[:], 0.0)
        for w in range(24):
            wps = psumS.tile([P, 2, QT_], f32, tag="st", name=f"w{w}")
            nc.tensor.matmul(wps[:, 0, :], wscr[:, 0:P], wscr[:],
                             start=True, stop=True)

        # ---------- emission units (generators yield every ~2 matmuls) ----
        emitted = set()

        def kq_gen(i, tt):
            """K and Q projections for pair i, token chunk tt (16 MMs)."""
            for (w_sb, dst, bcol) in ((wk_sb, KT_sb, 4 + i), (wq_sb, QT_sb, i)):
                ps = psumP.tile([P, QT_], f32, tag="proj",
                                name=f"pp{bcol}{i}{tt}")
                for k in range(8):
                    nc.tensor.matmul(ps[:], w_sb[:, i, k, :],
                                     xT_sb[:, tt, k, :],
                                     start=(k == 0), stop=(k == 7))
                    if k % 2 == 1 and k < 7:
                        yield
                nc.vector.tensor_scalar_add(dst[:, i, ds(QT_ * tt, QT_)],
                                            ps[:], bqk_sb[:, bcol : bcol + 1])
                yield
            emitted.add(("kq", i, tt))

        def v_gen(j):
            """V projection for kv block j, all 8 heads (8 MMs)."""
            ps = psumP.tile([P, HC], f32, tag="proj", name=f"pv{j}")
            for k in range(8):
                nc.tensor.matmul(ps[:],
                                 xT_sb[:, j // 4, k, ds(P * (j % 4), P)],
                                 wv_sb[:, k, :],
                                 start=(k == 0), stop=(k == 7))
                if k % 2 == 1 and k < 7:
                    yield
            nc.vector.tensor_copy(V_sb[:, j, :, 0:D],
                                  ps.rearrange("p (h d) -> p h d", d=D))
            emitted.add(("v", j))
            yield

        def o_gen(ko, tt):
            """Partial output projection rows 128ko, token chunk tt (4 MMs)."""
            ps = psumP.tile([P, QT_], f32, tag="proj", name=f"po{ko}{tt}")
            for k in range(4):
                nc.tensor.matmul(ps[:], wo_sb[:, k, ts(ko, P)],
                                 ctxT_sb[:, k, ds(QT_ * tt, QT_)],
                                 start=(k == 0), stop=(k == 3))
                if k == 1:
                    yield
            o_sb = opool.tile([P, QT_], bf16, tag="o", name=f"o{ko}{tt}")
            nc.vector.tensor_copy(o_sb[:], ps[:])
            eng = (nc.sync, nc.gpsimd)[(ko + 8 * tt) % 2]
            eng.dma_start(out[ts(ko, P), ds(QT_ * tt, QT_)], o_sb[:])
            yield

        # ---------- filler scheduler ----------
        FQ = []
        for i in (1, 2, 3):
            FQ.append(("kq", i, 0))
        FQ += [("v", 4), ("v", 5)]
        for i in range(4):
            FQ.append(("kq", i, 1))
        FQ += [("v", 6), ("v", 7)]
        for i in range(4):
            FQ.append(("kq", i, 2))
        FQ += [("v", 8), ("v", 9), ("v", 10), ("v", 11)]
        for i in range(4):
            FQ.append(("kq", i, 3))
        FQ += [("v", 12), ("v", 13), ("v", 14), ("v", 15)]
        FQ += [("o", ko, 0) for ko in range(8)]
        FQ += [("o", ko, 1) for ko in range(8)]
        FQ += [("o", ko, 2) for ko in range(4)]
        # o ko=4..8 of chunk 2 are reserved to fill the tail epilogue window
        reserve = [("o", ko, 2) for ko in range(4, 8)]

        def make_gen(key):
            if key[0] == "kq":
                return kq_gen(key[1], key[2])
            if key[0] == "v":
                return v_gen(key[1])
            return o_gen(key[1], key[2])

        state = {"cur": None, "curkey": None}
        ctx_done = set()  # q-chunks whose ctxT is fully written (stage B out)

        def next_key():
            for idx, key in enumerate(FQ):
                if key[0] == "o" and key[2] not in ctx_done:
                    continue  # ctxT for that chunk not complete yet
                FQ.pop(idx)
                return key
            return None

        def pump(steps):
            while steps > 0:
                if state["cur"] is None:
                    key = next_key()
                    if key is None:
                        return
                    state["cur"] = make_gen(key)
                    state["curkey"] = key
                try:
                    next(state["cur"])
                    steps -= 1
                except StopIteration:
                    state["cur"] = None
                    state["curkey"] = None

        def drain():
            if state["cur"] is not None:
                for _ in state["cur"]:
                    pass
                state["cur"] = None
                state["curkey"] = None

        def force(key):
            if key in emitted:
                return
            drain()
            if key in emitted:
                return  # the drained in-flight unit was this key
            if key in FQ:
                FQ.remove(key)
            for _ in make_gen(key):
                pass

        # ---------- attention ----------
        def scores(qc, i, j):
            st = psumS.tile([P, 2, QT_], f32, tag="st", name=f"st{qc}{i}{j}")
            for hh in range(2):
                nc.tensor.matmul(
                    st[:, hh, :],
                    KT_sb[ds(D * hh, D), i, ts(j, P)],
                    QT_sb[ds(D * hh, D), i, ds(QT_ * qc, QT_)],
                    start=True, stop=True)
            return st

        def exp_mask(qc, i, j, st):
            pt = ptpool.tile([P, 2, QT_], bf16, tag="pt", name=f"pt{qc}{i}{j}")
            nc.scalar.activation(pt[:], st[:], EXP)
            jl = j - 4 * qc
            if jl >= 0:
                nc.vector.tensor_tensor(
                    pt[:], pt[:],
                    mask_sb[:, jl : jl + 1, :].to_broadcast((P, 2, QT_)), MUL)
            return pt

        def ctx_mm(i, j, jmax, ctx_ps, pt):
            for hh in range(2):
                nc.tensor.matmul(
                    ctx_ps[0 : D + 1, hh, :],
                    V_sb[:, j, 2 * i + hh, :],
                    pt[:, hh, :],
                    start=(j == 0), stop=(j == jmax))

        deferred = []

        def epilogue_a(qc, i, ctx_ps):
            """Drain the pair's PSUM, stage l rows, 1/l; defer the PE/DVE
            normalization (stage B) so its wait doesn't bubble the PE."""
            cs = cspool.tile([D + 1, 2, QT_], f32, tag="cs", name=f"cs{qc}{i}")
            nc.vector.tensor_copy(cs[:], ctx_ps[0 : D + 1, :, :])
            l2 = lpool.tile([2, QT_], f32, tag="l", name=f"l{qc}{i}")
            nc.gpsimd.dma_start(l2[:], cs[D : D + 1, :, :])
            li = lipool.tile([2, QT_], f32, tag="li", name=f"li{qc}{i}")
            nc.vector.reciprocal_approx_fast(li[:], l2[:])
            lb = lbpool.tile([2, QT_], bf16, tag="lb", name=f"lb{qc}{i}")
            nc.vector.tensor_copy(lb[:], li[:])
            deferred.append((qc, i, cs, lb))

        def epilogue_b():
            while deferred:
                qc, i, cs, lb = deferred.pop(0)
                bct = psumS.tile([P, 2, QT_], f32, tag="st", name=f"bc{qc}{i}")
                for hh in range(2):
                    bc = bct[0:D, hh, :]
                    nc.tensor.matmul(bc, sel_sb[0:2, hh, :], lb[:],
                                     start=True, stop=True)
                    if hh == 0:
                        nc.vector.tensor_tensor(
                            ctxT_sb[0:D, i, ds(QT_ * qc, QT_)],
                            cs[0:D, 0, :], bc, MUL)
                    else:
                        sb = sbpool.tile([D, QT_], bf16, tag="sb",
                                         name=f"sb{qc}{i}")
                        nc.vector.tensor_tensor(sb[:], cs[0:D, 1, :], bc, MUL)
                        nc.gpsimd.dma_start(
                            ctxT_sb[ds(D, D), i, ds(QT_ * qc, QT_)], sb[:])
                if i == 3:
                    ctx_done.add(qc)

        # ---------- emission schedule ----------
        force(("kq", 0, 0))
        for j in range(4):
            force(("v", j))

        # Filler pacing: steps each qc's groups must emit so the next qc's
        # K/Q/V are projected before it starts (else forced PE bursts starve
        # ACT at qc boundaries).
        RATE = {0: 5.0, 1: 2.7, 2: 2.8, 3: 2.2}
        pairs = [(qc, i) for qc in range(NQC) for i in range(4)]
        pump_acc = 0.0
        carried = None
        carried_key = None
        for idx, (qc, i) in enumerate(pairs):
            jmax = (qc + 1) * 4 - 1
            force(("kq", i, qc))
            for j in range(jmax + 1):
                if ("v", j) not in emitted:
                    force(("v", j))
            ctx_ps = psumX.tile([P, 2, QT_], f32, tag="ctx",
                                name=f"ctx{qc}{i}")
            # kv blocks in groups of 2: one [sc,sc] burst per group keeps
            # row-grouped score MMs contiguous (each boundary between the
            # 64-contract score pairs and full-array MMs costs ~105ns of
            # exposed LDWEIGHTS).
            if carried_key == (qc, i):
                stg = carried
            else:
                stg = [scores(qc, i, 0), scores(qc, i, 1)]
            carried = None
            carried_key = None
            # flush the deferred 1/l work two groups later in the long
            # (ACT-bound) phases: their slots are shorter, so g==2 lands
            # ~0.6us before the recip chain is done
            flush_g = 4 if qc >= 2 else 2
            for g in range(0, jmax + 1, 2):
                if g == flush_g:
                    epilogue_b()
                nxt = ([scores(qc, i, g + 2), scores(qc, i, g + 3)]
                       if g + 2 <= jmax else None)
                pt0 = exp_mask(qc, i, g, stg[0])
                pt1 = exp_mask(qc, i, g + 1, stg[1])
                pump_acc += RATE[qc]
                take = int(pump_acc)
                pump_acc -= take
                pump(take)
                if g == jmax - 1 and idx + 1 < len(pairs):
                    nqc, ni = pairs[idx + 1]
                    if ("kq", ni, nqc) in emitted:
                        # pre-emit the next pair's first score group so ACT
                        # never idles across the pair boundary
                        carried = [scores(nqc, ni, 0), scores(nqc, ni, 1)]
                        carried_key = (nqc, ni)
                ctx_mm(i, g, jmax, ctx_ps, pt0)
                ctx_mm(i, g + 1, jmax, ctx_ps, pt1)
                stg = nxt
            epilogue_a(qc, i, ctx_ps)
        drain()
        # leftover + reserved fillers keep the PE busy while the last pair's
        # 1/l chain (cs copy -> hop -> recip -> cast) completes
        while FQ:
            key = FQ.pop(0)
            for _ in make_gen(key):
                pass
        for key in reserve:
            for _ in make_gen(key):
                pass
        epilogue_b()
        for ko in range(8):
            for _ in o_gen(ko, 3):
                pass

    nc.compile()
    return nc


def _shard_inputs(x, Wq, bq, Wk, bk, Wv, Wo):
    bf = ml_dtypes.bfloat16

    def kchunks(wT_cols):  # (1024, 512) -> [p, ko, k, u] (128, 4, 8, 128)
        return np.ascontiguousarray(
            wT_cols.reshape(8, P, 4, P).transpose(1, 2, 0, 3))

    wqT = (Wq.T / 8.0).astype(bf)
    wkT = Wk.T.astype(bf)
    wvT = Wv.T.astype(bf)
    woT = Wo.T.astype(bf)
    kv = np.arange(QT_)
    mask_np = np.ascontiguousarray(
        (kv[:, None] <= kv[None, :]).astype(bf)
        .reshape(4, P, QT_).transpose(1, 0, 2))
    sel_np = np.zeros((2, 2, D), dtype=bf)
    sel_np[0, 0, :] = 1.0
    sel_np[1, 1, :] = 1.0
    in_maps = []
    for b in range(B):
        xT = x[b].T.astype(bf)  # (1024, 2048)
        # [128k+p, 512tt+u] -> [p, tt, k, u]
        xp = np.ascontiguousarray(
            xT.reshape(8, P, 4, QT_).transpose(1, 2, 0, 3))
        for h in range(2):
            cols = slice(HC * h, HC * (h + 1))
            bq_h = (bq[cols] / 8.0).reshape(4, P).T
            bk_h = bk[cols].reshape(4, P).T
            bqk_np = np.ascontiguousarray(
                np.concatenate([bq_h, bk_h], axis=1)).astype(np.float32)
            in_maps.append({
                "xT": xp,
                "wqT": kchunks(wqT[:, cols]),
                "wkT": kchunks(wkT[:, cols]),
                "wvT": np.ascontiguousarray(
                    wvT[:, cols].reshape(8, P, HC)[:, :, :]
                    .transpose(1, 0, 2)),
                "woT": np.ascontiguousarray(
                    woT[cols, :].reshape(4, P, C).transpose(1, 0, 2)),
                "bqk": bqk_np,
                "mask": mask_np,
                "sel": sel_np,
            })
    return in_maps


def kernel(x, Wq, bq, Wk, bk, Wv, bv, Wo, bo):
    from concourse.bass_utils import run_bass_kernel_spmd

    x = np.asarray(x, np.float32)
    Wq = np.asarray(Wq, np.float32); bq = np.asarray(bq, np.float32)
    Wk = np.asarray(Wk, np.float32); bk = np.asarray(bk, np.float32)
    Wv = np.asarray(Wv, np.float32); bv = np.asarray(bv, np.float32)
    Wo = np.asarray(Wo, np.float32); bo = np.asarray(bo, np.float32)

    if "nc" not in _CACHE:
        _CACHE["nc"] = _build()
    nc = _CACHE["nc"]

    in_maps = _shard_inputs(x, Wq, bq, Wk, bk, Wv, Wo)
    res = run_bass_kernel_spmd(nc, in_maps, core_ids=list(range(8)))

    bias = (bo + Wo @ bv).astype(np.float32)
    outf = np.empty((B, T, C), np.float32)
    for b in range(B):
        p0 = np.asarray(res.results[2 * b]["out"], dtype=np.float32)
        p1 = np.asarray(res.results[2 * b + 1]["out"], dtype=np.float32)
        outf[b] = (p0 + p1).T + bias[None, :]
    return outf


# revision 4
# speedup vs baseline: 1.0280x; 1.0050x over previous
"""Causal multi-head attention (B=4, T=2048, C=1024, 16 heads) on 8 TRN2 cores.

Sharding v2: core (b, h) = (batch b, head-half h).  Each core projects
Q/K/V for its 8 heads only (no cross-core K/V redundancy), runs causally
tiled attention (q tiles of 512, kv extent (qc+1)*512 -- no fully-masked
tiles are ever computed), computes the partial output projection over its
512 ctx features, and the host sums the two partials per batch during the
unshard (out = p0 + p1 + bo + Wo@bv; bv is folded out via softmax rows
summing to 1, so the device never needs any V/O bias).

On-device layout is transposed ([feature, token]) like v1; softmax
denominators come from a ones-column appended to V.  Fixes vs v1:
  * reciprocal: [2,512] reciprocal_approx_fast per pair (was 32x 3.3us
    single-lane [1,512] full reciprocals = 106us DVE),
  * 1/l partition-broadcast via a tiny selector matmul on the PE instead
    of GpSimd partition_broadcast,
  * projection bias epilogues on DVE tensor_scalar (ACT does only exp),
  * all weights resident in SBUF (no per-matmul weight DMAs),
  * software-pipelined emission: scores(j+1) and 2-matmul projection
    filler chunks are emitted *before* ctx(j) so the PE never stalls
    behind the exp->mask chain, and the pair epilogue's PE work is
    deferred into the next pair to avoid pipeline bubbles.
"""

import numpy as np
import ml_dtypes

B, T, C, NH, D = 4, 2048, 1024, 16, 64
P = 128
HC = 512            # channels per head-half (8 heads x 64)
QT_ = 512           # q tile size
NQC = T // QT_      # 4 q tiles
NKV = T // P        # 16 kv blocks

_CACHE = {}


def _build():
    import concourse.bacc as bacc
    import concourse.tile as tile
    import concourse.mybir as mybir
    from concourse.bass import ts, ds

    f32 = mybir.dt.float32
    bf16 = mybir.dt.bfloat16
    EXP = mybir.ActivationFunctionType.Exp
    MUL = mybir.AluOpType.mult

    nc = bacc.Bacc("TRN2", target_bir_lowering=False, debug=False, num_devices=8)

    def din(name, shape, dt=bf16):
        return nc.dram_tensor(name, list(shape), dt, kind="ExternalInput").ap()

    # All inputs come pre-arranged on the host so every DMA slice is
    # contiguous per partition (strided (k p)->p k views halve the ring
    # throughput, which gated the prologue).
    x_v = din("xT", (P, 4, 8, QT_))   # x[b]^T as [p, tt, k, u]
    wq_v = din("wqT", (P, 4, 8, P))   # (Wq^T/8) cols_h as [p, ko, k, u]
    wk_v = din("wkT", (P, 4, 8, P))
    wv_v = din("wvT", (P, 8, HC))     # moving operand, loaded whole
    wo_v = din("woT", (P, 4, C))      # Wo^T rows of this half
    bqk = din("bqk", (P, 8), f32)     # cols 0:4 bq/8 chunks, 4:8 bk chunks
    mask_v = din("mask", (P, 4, QT_))  # tril block pattern as [p, jl, q]
    sel = din("sel", (2, 2, D))       # selector for 1/l broadcast matmul
    out = nc.dram_tensor("out", [C, T], bf16, kind="ExternalOutput").ap()

    from contextlib import ExitStack
    with ExitStack() as ctx:
        tc = ctx.enter_context(tile.TileContext(nc))

        consts = ctx.enter_context(tc.tile_pool(name="consts", bufs=1))
        big = ctx.enter_context(tc.tile_pool(name="big", bufs=1))
        ptpool = ctx.enter_context(tc.tile_pool(name="pt", bufs=4))
        cspool = ctx.enter_context(tc.tile_pool(name="cs", bufs=3))
        lpool = ctx.enter_context(tc.tile_pool(name="l", bufs=2))
        lipool = ctx.enter_context(tc.tile_pool(name="li", bufs=2))
        lbpool = ctx.enter_context(tc.tile_pool(name="lb", bufs=2))
        sbpool = ctx.enter_context(tc.tile_pool(name="sb", bufs=2))
        opool = ctx.enter_context(tc.tile_pool(name="o", bufs=3))
        psumP = ctx.enter_context(tc.tile_pool(name="psumP", bufs=2, space="PSUM"))
        psumS = ctx.enter_context(tc.tile_pool(name="psumS", bufs=2, space="PSUM"))
        psumX = ctx.enter_context(tc.tile_pool(name="psumX", bufs=1, space="PSUM"))

        # ---- resident tiles ----
        bqk_sb = consts.tile([P, 8], f32)
        sel_sb = consts.tile([2, 2, D], bf16)
        mask_sb = consts.tile([P, 4, QT_], bf16)
        xT_sb = big.tile([P, 4, 8, QT_], bf16)
        wq_sb = big.tile([P, 4, 8, P], bf16)
        wk_sb = big.tile([P, 4, 8, P], bf16)
        wv_sb = big.tile([P, 8, HC], bf16)
        wo_sb = big.tile([P, 4, C], bf16)
        KT_sb = big.tile([P, 4, T], bf16)
        QT_sb = big.tile([P, 4, T], bf16)
        V_sb = big.tile([P, NKV, 8, D + 1], bf16)
        ctxT_sb = big.tile([P, 4, T], bf16)

        # Input DMAs, deadline-sorted across three rings.  The sync ring
        # starts transfers ~4us before the scalar/gpsimd rings, so the
        # earliest-needed tensors (wk/wq ko=0, x tt=0, bqk, mask) go there.
        nc.sync.dma_start(wk_sb[:, 0, :, :], wk_v[:, 0, :, :])
        nc.sync.dma_start(xT_sb[:, 0, 0:4, :], x_v[:, 0, 0:4, :])
        nc.sync.dma_start(xT_sb[:, 0, 4:8, :], x_v[:, 0, 4:8, :])
        nc.sync.dma_start(wq_sb[:, 0, :, :], wq_v[:, 0, :, :])
        nc.sync.dma_start(bqk_sb[:], bqk)
        nc.sync.dma_start(mask_sb[:], mask_v)
        nc.sync.dma_start(sel_sb[:], sel)
        for tt in range(1, 4):
            nc.sync.dma_start(xT_sb[:, tt, :, :], x_v[:, tt, :, :])
        nc.scalar.dma_start(wv_sb[:], wv_v)
        nc.scalar.dma_start(wo_sb[:], wo_v)
        for ko in range(1, 4):
            nc.gpsimd.dma_start(wk_sb[:, ko, :, :], wk_v[:, ko, :, :])
            nc.gpsimd.dma_start(wq_sb[:, ko, :, :], wq_v[:, ko, :, :])

        nc.vector.memset(V_sb[:, :, :, D : D + 1], 1.0)
        # preload the exp table set during the DMA prologue
        dscr = consts.tile([1, 8], f32)
        dout = consts.tile([1, 8], f32)
        nc.vector.memset(dscr[:], 0.0)
        nc.scalar.activation(dout[:], dscr[:], EXP)
        # HAM warm-up: ~4us of dummy matmuls while the input DMAs land, so
        # the PE clock is at 2.4GHz (not the cold 1.2) when real work starts.
        wscr = consts.tile([P, QT_], bf16)
        nc.vector.memset(wscr[:], 0.0)
        for w in range(24):
            wps = psumS.tile([P, 2, QT_], f32, tag="st", name=f"w{w}")
            nc.tensor.matmul(wps[:, 0, :], wscr[:, 0:P], wscr[:],
                             start=True, stop=True)

        # ---------- emission units (generators yield every ~2 matmuls) ----
        emitted = set()

        def kq_gen(i, tt):
            """K and Q projections for pair i, token chunk tt (16 MMs)."""
            for (w_sb, dst, bcol) in ((wk_sb, KT_sb, 4 + i), (wq_sb, QT_sb, i)):
                ps = psumP.tile([P, QT_], f32, tag="proj",
                                name=f"pp{bcol}{i}{tt}")
                for k in range(8):
                    nc.tensor.matmul(ps[:], w_sb[:, i, k, :],
                                     xT_sb[:, tt, k, :],
                                     start=(k == 0), stop=(k == 7))
                    if k % 2 == 1 and k < 7:
                        yield
                nc.vector.tensor_scalar_add(dst[:, i, ds(QT_ * tt, QT_)],
                                            ps[:], bqk_sb[:, bcol : bcol + 1])
                yield
            emitted.add(("kq", i, tt))

        def v_gen(j):
            """V projection for kv block j, all 8 heads (8 MMs)."""
            ps = psumP.tile([P, HC], f32, tag="proj", name=f"pv{j}")
            for k in range(8):
                nc.tensor.matmul(ps[:],
                                 xT_sb[:, j // 4, k, ds(P * (j % 4), P)],
                                 wv_sb[:, k, :],
                                 start=(k == 0), stop=(k == 7))
                if k % 2 == 1 and k < 7:
                    yield
            nc.vector.tensor_copy(V_sb[:, j, :, 0:D],
                                  ps.rearrange("p (h d) -> p h d", d=D))
            emitted.add(("v", j))
            yield

        def o_gen(ko, tt):
            """Partial output projection rows 128ko, token chunk tt (4 MMs)."""
            ps = psumP.tile([P, QT_], f32, tag="proj", name=f"po{ko}{tt}")
            for k in range(4):
                nc.tensor.matmul(ps[:], wo_sb[:, k, ts(ko, P)],
                                 ctxT_sb[:, k, ds(QT_ * tt, QT_)],
                                 start=(k == 0), stop=(k == 3))
                if k == 1:
                    yield
            o_sb = opool.tile([P, QT_], bf16, tag="o", name=f"o{ko}{tt}")
            nc.vector.tensor_copy(o_sb[:], ps[:])
            eng = (nc.sync, nc.gpsimd)[(ko + 8 * tt) % 2]
            eng.dma_start(out[ts(ko, P), ds(QT_ * tt, QT_)], o_sb[:])
            yield

        # ---------- filler scheduler ----------
        FQ = []
        for i in (1, 2, 3):
            FQ.append(("kq", i, 0))
        FQ += [("v", 4), ("v", 5)]
        for i in range(4):
            FQ.append(("kq", i, 1))
        FQ += [("v", 6), ("v", 7)]
        for i in range(4):
            FQ.append(("kq", i, 2))
        FQ += [("v", 8), ("v", 9), ("v", 10), ("v", 11)]
        for i in range(4):
            FQ.append(("kq", i, 3))
        FQ += [("v", 12), ("v", 13), ("v", 14), ("v", 15)]
        FQ += [("o", ko, 0) for ko in range(8)]
        FQ += [("o", ko, 1) for ko in range(8)]
        FQ += [("o", ko, 2) for ko in range(4)]
        # o ko=4..8 of chunk 2 are reserved to fill the tail epilogue window
        reserve = [("o", ko, 2) for ko in range(4, 8)]

        def make_gen(key):
            if key[0] == "kq":
                return kq_gen(key[1], key[2])
            if key[0] == "v":
                return v_gen(key[1])
            return o_gen(key[1], key[2])

        state = {"cur": None, "curkey": None}
        ctx_done = set()  # q-chunks whose ctxT is fully written (stage B out)

        def next_key():
            for idx, key in enumerate(FQ):
                if key[0] == "o" and key[2] not in ctx_done:
                    continue  # ctxT for that chunk not complete yet
                FQ.pop(idx)
                return key
            return None

        def pump(steps):
            while steps > 0:
                if state["cur"] is None:
                    key = next_key()
                    if key is None:
                        return
                    state["cur"] = make_gen(key)
                    state["curkey"] = key
                try:
                    next(state["cur"])
                    steps -= 1
                except StopIteration:
                    state["cur"] = None
                    state["curkey"] = None

        def drain():
            if state["cur"] is not None:
                for _ in state["cur"]:
                    pass
                state["cur"] = None
                state["curkey"] = None

        def force(key):
            if key in emitted:
                return
            drain()
            if key in emitted:
                return  # the drained in-flight unit was this key
            if key in FQ:
                FQ.remove(key)
            for _ in make_gen(key):
                pass

        # ---------- attention ----------
        def scores(qc, i, j):
            st = psumS.tile([P, 2, QT_], f32, tag="st", name=f"st{qc}{i}{j}")
            for hh in range(2):
                nc.tensor.matmul(
                    st[:, hh, :],
                    KT_sb[ds(D * hh, D), i, ts(j, P)],
                    QT_sb[ds(D * hh, D), i, ds(QT_ * qc, QT_)],
                    start=True, stop=True)
            return st

        def exp_mask(qc, i, j, st):
            pt = ptpool.tile([P, 2, QT_], bf16, tag="pt", name=f"pt{qc}{i}{j}")
            nc.scalar.activation(pt[:], st[:], EXP)
            jl = j - 4 * qc
            if jl >= 0:
                nc.vector.tensor_tensor(
                    pt[:], pt[:],
                    mask_sb[:, jl : jl + 1, :].to_broadcast((P, 2, QT_)), MUL)
            return pt

        def ctx_mm(i, j, jmax, ctx_ps, pt):
            for hh in range(2):
                nc.tensor.matmul(
                    ctx_ps[0 : D + 1, hh, :],
                    V_sb[:, j, 2 * i + hh, :],
                    pt[:, hh, :],
                    start=(j == 0), stop=(j == jmax))

        deferred = []
        last_tiles = {}

        def epilogue_a(qc, i, ctx_ps):
            """Drain the pair's PSUM, stage l rows, 1/l; defer the PE/DVE
            normalization (stage B) so its wait doesn't bubble the PE."""
            cs = cspool.tile([D + 1, 2, QT_], f32, tag="cs", name=f"cs{qc}{i}")
            nc.vector.tensor_copy(cs[:], ctx_ps[0 : D + 1, :, :])
            l2 = lpool.tile([2, QT_], f32, tag="l", name=f"l{qc}{i}")
            nc.gpsimd.dma_start(l2[:], cs[D : D + 1, :, :])
            li = lipool.tile([2, QT_], f32, tag="li", name=f"li{qc}{i}")
            nc.vector.reciprocal_approx_fast(li[:], l2[:])
            lb = lbpool.tile([2, QT_], bf16, tag="lb", name=f"lb{qc}{i}")
            nc.vector.tensor_copy(lb[:], li[:])
            deferred.append((qc, i, cs, lb))

        def epilogue_b():
            while deferred:
                qc, i, cs, lb = deferred.pop(0)
                bct = psumS.tile([P, 2, QT_], f32, tag="st", name=f"bc{qc}{i}")
                for hh in range(2):
                    bc = bct[0:D, hh, :]
                    nc.tensor.matmul(bc, sel_sb[0:2, hh, :], lb[:],
                                     start=True, stop=True)
                    if hh == 0:
                        nc.vector.tensor_tensor(
                            ctxT_sb[0:D, i, ds(QT_ * qc, QT_)],
                            cs[0:D, 0, :], bc, MUL)
                    else:
                        sb = sbpool.tile([D, QT_], bf16, tag="sb",
                                         name=f"sb{qc}{i}")
                        nc.vector.tensor_tensor(sb[:], cs[0:D, 1, :], bc, MUL)
                        nc.gpsimd.dma_start(
                            ctxT_sb[ds(D, D), i, ds(QT_ * qc, QT_)], sb[:])
                        last_tiles["sb"] = sb
                if i == 3:
                    ctx_done.add(qc)

        # ---------- emission schedule ----------
        force(("kq", 0, 0))
        for j in range(4):
            force(("v", j))

        # Filler pacing: steps each qc's groups must emit so the next qc's
        # K/Q/V are projected before it starts (else forced PE bursts starve
        # ACT at qc boundaries).
        RATE = {0: 5.0, 1: 2.7, 2: 2.8, 3: 2.2}
        pairs = [(qc, i) for qc in range(NQC) for i in range(4)]
        pump_acc = 0.0
        carried = None
        carried_key = None
        for idx, (qc, i) in enumerate(pairs):
            jmax = (qc + 1) * 4 - 1
            force(("kq", i, qc))
            for j in range(jmax + 1):
                if ("v", j) not in emitted:
                    force(("v", j))
            ctx_ps = psumX.tile([P, 2, QT_], f32, tag="ctx",
                                name=f"ctx{qc}{i}")
            # kv blocks in groups of 2: one [sc,sc] burst per group keeps
            # row-grouped score MMs contiguous (each boundary between the
            # 64-contract score pairs and full-array MMs costs ~105ns of
            # exposed LDWEIGHTS).
            if carried_key == (qc, i):
                stg = carried
            else:
                stg = [scores(qc, i, 0), scores(qc, i, 1)]
            carried = None
            carried_key = None
            # flush the deferred 1/l work two groups later in the long
            # (ACT-bound) phases: their slots are shorter, so g==2 lands
            # ~0.6us before the recip chain is done
            flush_g = 4 if qc >= 2 else 2
            for g in range(0, jmax + 1, 2):
                if g == flush_g:
                    epilogue_b()
                nxt = ([scores(qc, i, g + 2), scores(qc, i, g + 3)]
                       if g + 2 <= jmax else None)
                pt0 = exp_mask(qc, i, g, stg[0])
                pt1 = exp_mask(qc, i, g + 1, stg[1])
                pump_acc += RATE[qc]
                take = int(pump_acc)
                pump_acc -= take
                pump(take)
                if g == jmax - 1 and idx + 1 < len(pairs):
                    nqc, ni = pairs[idx + 1]
                    if ("kq", ni, nqc) in emitted:
                        # pre-emit the next pair's first score group so ACT
                        # never idles across the pair boundary
                        carried = [scores(nqc, ni, 0), scores(nqc, ni, 1)]
                        carried_key = (nqc, ni)
                ctx_mm(i, g, jmax, ctx_ps, pt0)
                ctx_mm(i, g + 1, jmax, ctx_ps, pt1)
                if qc == 3 and i == 3 and g + 1 == jmax:
                    last_tiles["pt"] = pt1
                stg = nxt
            epilogue_a(qc, i, ctx_ps)
        drain()
        # leftover + reserved fillers keep the PE busy while the last pair's
        # 1/l chain (cs copy -> hop -> recip -> cast) completes
        while FQ:
            key = FQ.pop(0)
            for _ in make_gen(key):
                pass
        for key in reserve:
            for _ in make_gen(key):
                pass
        # tail-pinned PE filler: dummies whose rhs depends on the last pair's
        # attention, so the scheduler cannot hoist them out of the window
        # where the final 1/l chain (copy->hop->recip->cast) runs
        for w in range(20):
            wps = psumS.tile([P, 2, QT_], f32, tag="st", name=f"tw{w}")
            nc.tensor.matmul(wps[:, 0, :], wscr[:, 0:P],
                             last_tiles["pt"][:, 0, :], start=True, stop=True)
        epilogue_b()
        for w in range(6):
            wps = psumS.tile([P, 2, QT_], f32, tag="st", name=f"uw{w}")
            nc.tensor.matmul(wps[0:P, 0, :], wscr[0:D, 0:P],
                             last_tiles["sb"][:], start=True, stop=True)
        for ko in range(8):
            for _ in o_gen(ko, 3):
                pass

    nc.compile()
    return nc


def _shard_inputs(x, Wq, bq, Wk, bk, Wv, Wo):
    bf = ml_dtypes.bfloat16

    def kchunks(wT_cols):  # (1024, 512) -> [p, ko, k, u] (128, 4, 8, 128)
        return np.ascontiguousarray(
            wT_cols.reshape(8, P, 4, P).transpose(1, 2, 0, 3))

    wqT = (Wq.T / 8.0).astype(bf)
    wkT = Wk.T.astype(bf)
    wvT = Wv.T.astype(bf)
    woT = Wo.T.astype(bf)
    kv = np.arange(QT_)
    mask_np = np.ascontiguousarray(
        (kv[:, None] <= kv[None, :]).astype(bf)
        .reshape(4, P, QT_).transpose(1, 0, 2))
    sel_np = np.zeros((2, 2, D), dtype=bf)
    sel_np[0, 0, :] = 1.0
    sel_np[1, 1, :] = 1.0
    in_maps = []
    for b in range(B):
        xT = x[b].T.astype(bf)  # (1024, 2048)
        # [128k+p, 512tt+u] -> [p, tt, k, u]
        xp = np.ascontiguousarray(
            xT.reshape(8, P, 4, QT_).transpose(1, 2, 0, 3))
        for h in range(2):
            cols = slice(HC * h, HC * (h + 1))
            bq_h = (bq[cols] / 8.0).reshape(4, P).T
            bk_h = bk[cols].reshape(4, P).T
            bqk_np = np.ascontiguousarray(
                np.concatenate([bq_h, bk_h], axis=1)).astype(np.float32)
            in_maps.append({
                "xT": xp,
                "wqT": kchunks(wqT[:, cols]),
                "wkT": kchunks(wkT[:, cols]),
                "wvT": np.ascontiguousarray(
                    wvT[:, cols].reshape(8, P, HC)[:, :, :]
                    .transpose(1, 0, 2)),
                "woT": np.ascontiguousarray(
                    woT[cols, :].reshape(4, P, C).transpose(1, 0, 2)),
                "bqk": bqk_np,
                "mask": mask_np,
                "sel": sel_np,
            })
    return in_maps


def kernel(x, Wq, bq, Wk, bk, Wv, bv, Wo, bo):
    from concourse.bass_utils import run_bass_kernel_spmd

    x = np.asarray(x, np.float32)
    Wq = np.asarray(Wq, np.float32); bq = np.asarray(bq, np.float32)
    Wk = np.asarray(Wk, np.float32); bk = np.asarray(bk, np.float32)
    Wv = np.asarray(Wv, np.float32); bv = np.asarray(bv, np.float32)
    Wo = np.asarray(Wo, np.float32); bo = np.asarray(bo, np.float32)

    if "nc" not in _CACHE:
        _CACHE["nc"] = _build()
    nc = _CACHE["nc"]

    in_maps = _shard_inputs(x, Wq, bq, Wk, bk, Wv, Wo)
    res = run_bass_kernel_spmd(nc, in_maps, core_ids=list(range(8)))

    bias = (bo + Wo @ bv).astype(np.float32)
    outf = np.empty((B, T, C), np.float32)
    for b in range(B):
        p0 = np.asarray(res.results[2 * b]["out"], dtype=np.float32)
        p1 = np.asarray(res.results[2 * b + 1]["out"], dtype=np.float32)
        outf[b] = (p0 + p1).T + bias[None, :]
    return outf
